# revision 3
# baseline (speedup 1.0000x reference)
"""Trainium2 Bass kernel for nn_BilinearPairedLayer.

out[b,i,j,o] = celu(zl[b,i] @ fc_l_W^T + fc_l_b) @ W[o] @ celu(zr[b,j] @ fc_r_W^T + fc_r_b) + bb[o]

with context-3 pairing:
  zl = [x_l, shift_fwd(x_l,1), shift_bwd(x_l,1)]   (192 features)
  zr = [x_l, shift_bwd(x_r,1), shift_fwd(x_r,1)]   (faithful torch-source bug: x_l first)

Shapes: B=2, N=512, n_in=64, H=128, n_out=8 -> out [2,512,512,8] f32.

Sharding: 8 cores = (b in {0,1}) x (j-chunk in {0..3} of 128 columns).
Each core computes out[b, :, j0:j0+128, :] (as bf16; host upcasts).

Per-core dataflow (contraction dims pre-transposed onto partitions host-side):
  D1a [128,662] bf16 (sync ring): cols 0:384 rows 0:64 = frT, rows 64:128
      = flT (overlay: each matmul's rhs has zeros in the other side's rows);
      384:512 xljT (rows 0:64); 512:642 xrhT; 642:646 fc biases (raw f32
      bytes, 2 bf16 cols each); 646:662 out-bias bb[c] broadcast per
      partition (8 f32 bitcast pairs).
  D1b [128,514] bf16 (sync ring): xlhT rows 64:128, zeros on top.
  Wt  [128,1024] bf16 (scalar ring): WT[g, o*H+h] = bilinear_W[o,h,g].

  0. PE warm-up: ~10 dummy matmuls on a memset tile while the input DMAs
     are in flight, so HAM un-throttles (1.2->2.4 GHz) before real work.
     A dummy Exp on ACT pulls the ~1.3us activation-table load early.
  1. hrT[g,j] = celu(sum_c frT_c.T @ {xljT, xrhT +-1})      3 matmuls
  2. hlT[h,i] = celu(sum_c flT_c.T @ xlhT_shift_c)          3 matmuls
     celu(x) = max(x+b, min(exp(x+b)-1, 0)); chain split across
     DVE/ACT/GpSimd (GpSimd cannot read PSUM, takes SBUF-side min/max).
  3. v2[h,(o,j)] : per o: WT_o.T @ hrT -> psum [h, o*128+j], two 512-col
     og groups, cast to bf16 contiguously (og0 on ACT, og1 on DVE).
  4. TRANSPOSED main: psum_c[j,i] = v2_c.T @ hlT (stationary = v2 chunk,
     moving = hlT, N=512).  Chunk c holds o=c for all (j,i), so the
     bilinear bias is a per-partition CONSTANT: eviction = one fused
     copy+bias+bf16-cast per chunk (ACT activation-with-bias / DVE
     tensor_scalar), alternating ACT/DVE.
  5. out DRAM [128, 4096] bf16: chunk c at cols c*512..(c+1)*512, i.e.
     out_d[j, c*512+i].  4 paired DMAs [128,1024] on the (idle) sync
     ring, issued as soon as each pair of chunks is evicted.  Host
     upcasts + transposes.

walrus's per-instruction HW structs carry at most ONE sync wait; a post-pass
splits multi-wait instructions into single-wait EventSemaphore predecessors.
"""

import numpy as np

import concourse.bass as bass
import concourse.mybir as mybir
import concourse.tile as tile
from concourse.bass_utils import run_bass_kernel_spmd

F32 = mybir.dt.float32
BF16 = mybir.dt.bfloat16

B = 2
N = 512
NIN = 64
H = 128
O = 8
JC = 128  # j-chunk per core
N_CORES = 8

# D1a packed-column offsets (bf16 elements; f32 biases take 2 cols each)
_W0 = 0               # frT (rows 0:64) / flT (rows 64:128)  [3*128]
_XLJ = 384            # xljT  [128]   (rows 0:64)
_XRH = 512            # xrhT  [130]   (rows 0:64)
_BL = 642             # fc_l_b (f32 bytes)
_BR = 644             # fc_r_b
_OB = 646             # bilinear_b[c] broadcast, 8 x f32 bytes
_D1AW = 662
_D1BW = 514           # xlhT (rows 64:128)

N_WARM = 10


def build_nc():
    nc = bass.Bass("TRN2")

    D1a = nc.dram_tensor("D1a", [128, _D1AW], BF16, kind="ExternalInput")
    D1b = nc.dram_tensor("D1b", [128, _D1BW], BF16, kind="ExternalInput")
    Wt = nc.dram_tensor("Wt", [128, O * H], BF16, kind="ExternalInput")
    out_d = nc.dram_tensor("out", [128, O * N], BF16, kind="ExternalOutput")

    with tile.TileContext(nc) as tc:
        with (
            tc.tile_pool(name="persist", bufs=1) as pp,
            tc.tile_pool(name="ps_l1", bufs=1, space="PSUM") as ps_l1,
            tc.tile_pool(name="ps_v", bufs=1, space="PSUM") as ps_v,
            tc.tile_pool(name="ps_main", bufs=4, space="PSUM") as ps_main,
        ):
            D1a_sb = pp.tile([128, _D1AW], BF16, name="D1a_sb")
            D1b_sb = pp.tile([128, _D1BW], BF16, name="D1b_sb")
            W_sb = pp.tile([128, O * H], BF16, name="W_sb")
            warm_sb = pp.tile([128, 384], BF16, name="warm_sb")
            td = pp.tile([1, 2], F32, name="td")

            # ---- input DMAs: D1a/D1b on sync ring, Wt on scalar ring ----
            nc.sync.dma_start(D1a_sb[:], D1a[:])
            nc.sync.dma_start(D1b_sb[:], D1b[:])
            nc.scalar.dma_start(W_sb[:], Wt[:])

            # ---- PE warm-up + early ACT table load ----
            nc.vector.memset(warm_sb[:], 0.0)
            nc.vector.memset(td[0:1, 0:1], 0.0)
            nc.scalar.activation(td[0:1, 1:2], td[0:1, 0:1],
                                 mybir.ActivationFunctionType.Exp)

            ps_hr = ps_l1.tile([128, JC], F32, name="ps_hr")
            ps_hl = ps_l1.tile([128, N], F32, name="ps_hl")
            for _ in range(N_WARM):
                nc.tensor.matmul(
                    ps_hl[:, 0:256], warm_sb[:, 0:128], warm_sb[:, 128:384],
                    start=True, stop=True,
                )

            bl_ap = D1a_sb[:, _BL:_BL + 2].bitcast(F32)
            br_ap = D1a_sb[:, _BR:_BR + 2].bitcast(F32)

            # ---- layer 1 matmuls ----
            rhs_r = [
                D1a_sb[:, _XLJ:_XLJ + JC],           # x_l[j]
                D1a_sb[:, _XRH + 2:_XRH + 2 + JC],   # x_r[j+1] (bwd)
                D1a_sb[:, _XRH:_XRH + JC],           # x_r[j-1] (fwd)
            ]
            for c in range(3):
                nc.tensor.matmul(
                    ps_hr[:], D1a_sb[:, _W0 + c * H:_W0 + (c + 1) * H],
                    rhs_r[c], start=(c == 0), stop=(c == 2),
                )
            rhs_l = [
                D1b_sb[:, 1:1 + N],    # x_l[i]
                D1b_sb[:, 0:N],        # x_l[i-1] (fwd)
                D1b_sb[:, 2:2 + N],    # x_l[i+1] (bwd)
            ]
            for c in range(3):
                nc.tensor.matmul(
                    ps_hl[:], D1a_sb[:, _W0 + c * H:_W0 + (c + 1) * H],
                    rhs_l[c], start=(c == 0), stop=(c == 2),
                )

            # ---- hr celu (DVE + ACT): hrT[g,j] bf16 ----
            hrT = pp.tile([128, JC], BF16, name="hrT")
            pre_r = pp.tile([128, JC], F32, name="pre_r")
            e_r = pp.tile([128, JC], F32, name="e_r")
            nc.vector.tensor_scalar_add(pre_r[:], ps_hr[:], br_ap)
            nc.scalar.activation(e_r[:], ps_hr[:],
                                 mybir.ActivationFunctionType.Exp,
                                 bias=br_ap, scale=1.0)
            nc.vector.tensor_scalar(e_r[:], e_r[:], -1.0, 0.0,
                                    mybir.AluOpType.add, mybir.AluOpType.min)
            nc.vector.tensor_tensor(hrT[:], pre_r[:], e_r[:],
                                    mybir.AluOpType.max)

            # ---- v2 matmuls: psum [h, (o,j)] per og group ----
            ps_v0 = ps_v.tile([128, 512], F32, name="ps_v0")
            ps_v1 = ps_v.tile([128, 512], F32, name="ps_v1")
            for og, ps_vo in ((0, ps_v0), (1, ps_v1)):
                for ol in range(4):
                    o = og * 4 + ol
                    nc.tensor.matmul(
                        ps_vo[:, ol * JC:(ol + 1) * JC],
                        W_sb[:, o * H:(o + 1) * H], hrT[:],
                        start=True, stop=True,
                    )

            # ---- hl celu in halves: hlT[h,i] bf16 ----
            # GpSimd takes the SBUF-side min steps (Pool supports
            # tensor_scalar add/min but not tensor_tensor max).
            hlT = pp.tile([128, N], BF16, name="hlT")
            pre_l = pp.tile([128, N], F32, name="pre_l")
            e_l = pp.tile([128, N], F32, name="e_l")
            for ch in range(2):
                sl = slice(ch * 256, (ch + 1) * 256)
                nc.vector.tensor_scalar_add(pre_l[:, sl], ps_hl[:, sl], bl_ap)
                nc.scalar.activation(e_l[:, sl], ps_hl[:, sl],
                                     mybir.ActivationFunctionType.Exp,
                                     bias=bl_ap, scale=1.0)
                nc.gpsimd.tensor_scalar(e_l[:, sl], e_l[:, sl], -1.0, 0.0,
                                        mybir.AluOpType.add,
                                        mybir.AluOpType.min)
            for ch in range(2):
                sl = slice(ch * 256, (ch + 1) * 256)
                nc.vector.tensor_tensor(hlT[:, sl], pre_l[:, sl], e_l[:, sl],
                                        mybir.AluOpType.max)

            # ---- v2 casts to bf16 (contiguous; og0 ACT, og1 DVE) ----
            v2sb = pp.tile([128, O * H], BF16, name="v2sb")
            nc.scalar.copy(v2sb[:, 0:512], ps_v0[:])
            nc.vector.tensor_copy(v2sb[:, 512:1024], ps_v1[:])

            # ---- main (transposed): psum_c[j, i] = v2_c.T @ hlT ----
            out_sb = pp.tile([128, O * N], BF16, name="out_sb")
            ev_done = []
            for c in range(O):
                ps_m = ps_main.tile([128, N], F32, name="ps_m")
                nc.tensor.matmul(
                    ps_m[:], v2sb[:, c * JC:(c + 1) * JC], hlT[:],
                    start=True, stop=True,
                )
                ob_ap = D1a_sb[:, _OB + 2 * c:_OB + 2 * c + 2].bitcast(F32)
                dst = out_sb[:, c * N:(c + 1) * N]
                if c % 2 == 0:
                    nc.scalar.activation(dst, ps_m[:],
                                         mybir.ActivationFunctionType.Identity,
                                         bias=ob_ap, scale=1.0)
                else:
                    nc.vector.tensor_scalar_add(dst, ps_m[:], ob_ap)
                ev_done.append(dst)
                if c % 2 == 1:
                    p = c // 2
                    nc.sync.dma_start(
                        out_d[:, p * 1024:(p + 1) * 1024],
                        out_sb[:, p * 1024:(p + 1) * 1024])

    _legalize_waits(nc)
    return nc


def _legalize_waits(nc):
    """walrus's per-instruction HW structs carry at most ONE sync wait.
    Split any instruction with >1 on_wait into same-engine single-wait
    EventSemaphore predecessors (engine executes them in program order)."""
    n = 0
    for bb in nc.main_func.blocks:
        insts = list(bb.instructions)
        out = []
        for ins in insts:
            si = ins.sync_info
            waits = list(si.on_wait) if si and si.on_wait else []
            if len(waits) > 1:
                for w in waits[:-1]:
                    n += 1
                    out.append(mybir.InstEventSemaphore(
                        name=f"wait-split-{n}",
                        opcode="EventSemaphore",
                        engine=ins.engine,
                        ins=[], outs=[],
                        sync_info=mybir.SyncInfo(on_wait=[w], on_update=[]),
                    ))
                si.on_wait = [waits[-1]]
            out.append(ins)
        if n:
            bb.instructions = out
    return nc


_NC_CACHE = None


def _get_nc():
    global _NC_CACHE
    if _NC_CACHE is None:
        _NC_CACHE = build_nc()
    return _NC_CACHE


def _prep_core_inputs(x_l, x_r, fc_l_W, fc_l_b, fc_r_W, fc_r_b, bilinear_W, bilinear_b):
    """Host-side sharding: build the 8 per-core input dicts."""
    import ml_dtypes

    f32 = np.float32
    bf16 = ml_dtypes.bfloat16
    x_l = np.ascontiguousarray(x_l, f32)
    x_r = np.ascontiguousarray(x_r, f32)

    # WT[g, o*H + h] = W[o, h, g]
    WT = np.ascontiguousarray(
        np.asarray(bilinear_W, f32).transpose(2, 0, 1).reshape(128, O * H)
    ).astype(bf16)

    D1c = np.zeros((128, _D1AW), bf16)
    frW = np.asarray(fc_r_W, f32)
    flW = np.asarray(fc_l_W, f32)
    for c in range(3):
        D1c[:NIN, _W0 + c * H:_W0 + (c + 1) * H] = \
            frW[:, c * NIN:(c + 1) * NIN].T.astype(bf16)
        D1c[NIN:, _W0 + c * H:_W0 + (c + 1) * H] = \
            flW[:, c * NIN:(c + 1) * NIN].T.astype(bf16)
    # bias columns carry raw f32 bytes (two bf16 slots per value)
    u2 = D1c.view(np.uint16)
    u2[:, _BL:_BL + 2] = np.asarray(fc_l_b, f32).reshape(-1, 1).view('<u2')
    u2[:, _BR:_BR + 2] = np.asarray(fc_r_b, f32).reshape(-1, 1).view('<u2')
    bbu = np.asarray(bilinear_b, f32).view('<u2')  # [16]
    u2[:, _OB:_OB + 2 * O] = bbu.reshape(1, -1)

    # D1b per batch: xlhT rows 64:128, col t = x_l[b, t-1]
    D1bs = []
    for b in range(B):
        D1b = np.zeros((128, _D1BW), bf16)
        D1b[NIN:, 1:1 + N] = x_l[b].T.astype(bf16)
        D1bs.append(D1b)

    in_maps = []
    for core in range(N_CORES):
        b, jg = core // 4, core % 4
        j0 = jg * JC
        D1 = D1c.copy()
        D1[:NIN, _XLJ:_XLJ + JC] = x_l[b, j0:j0 + JC].T.astype(bf16)
        # xrhT: col t = x_r[b, j0-1+t], zero-padded at global edges
        lo = max(j0 - 1, 0)
        hi = min(j0 + JC + 1, N)
        D1[:NIN, _XRH + lo - (j0 - 1):_XRH + hi - (j0 - 1)] = \
            x_r[b, lo:hi].T.astype(bf16)
        in_maps.append({"D1a": D1, "D1b": D1bs[b], "Wt": WT})
    return in_maps


def _run(inputs, trace=False, **kw):
    nc = _get_nc()
    in_maps = _prep_core_inputs(**inputs)
    res = run_bass_kernel_spmd(
        nc, in_maps, core_ids=list(range(N_CORES)), trace=trace, **kw)
    out = np.empty((B, N, N, O), np.float32)
    for core in range(N_CORES):
        b, jg = core // 4, core % 4
        j0 = jg * JC
        # device out: [j, (o, i)] -> [i, j, o]
        arr = np.asarray(res.results[core]["out"]).astype(np.float32)
        out[b, :, j0:j0 + JC, :] = arr.reshape(JC, O, N).transpose(2, 0, 1)
    return out, res


def kernel(**inputs):
    out, _ = _run(inputs, trace=False)
    return out


# revision 5
# speedup vs baseline: 1.3339x; 1.3339x over previous
"""Trainium2 Bass kernel for nn_BilinearPairedLayer.

out[b,i,j,o] = celu(zl[b,i] @ fc_l_W^T + fc_l_b) @ W[o] @ celu(zr[b,j] @ fc_r_W^T + fc_r_b) + bb[o]

with context-3 pairing:
  zl = [x_l, shift_fwd(x_l,1), shift_bwd(x_l,1)]   (192 features)
  zr = [x_l, shift_bwd(x_r,1), shift_fwd(x_r,1)]   (faithful torch-source bug: x_l first)

Shapes: B=2, N=512, n_in=64, H=128, n_out=8 -> out [2,512,512,8] f32.

Sharding: 8 cores = (b in {0,1}) x (j-chunk in {0..3} of 128 columns).
Each core computes out[b, :, j0:j0+128, :] (as bf16; host upcasts).

Per-core dataflow (contraction dims pre-transposed onto partitions host-side).
Inputs ride THREE parallel DMA paths so descriptor generation overlaps:
  D1w [128,384] bf16 (sync):   rows 0:64 = frT, rows 64:128 = flT overlay
                               (each matmul's rhs is zero in the other rows)
  D1x [128,516] bf16 (scalar): xljT 0:128; xrhT 128:258; partition-0 rows
                               @258/@386 = fc_r_b / fc_l_b for the K=1 bias
                               matmul; 514:516 = out-bias bb[p%8] (bitcast)
  D1b [128,514] bf16 (gpsimd SWDGE): xlhT rows 64:128, zeros on top
  Wt  [128,1024] bf16 (sync, 2nd): WT[g, o*H+h] = bilinear_W[o,h,g]

  0. PE warm-up: dummy matmuls on a memset tile while input DMAs fly, so
     HAM un-throttles (1.2->2.4 GHz) before real work; a dummy Exp pulls
     the ~1.5us ACT table load early.
  1. fc biases are accumulated INTO PSUM by a K=1 rank-1 matmul
     (bias-row^T @ ones-row), so celu needs no separate pre-add:
     celu chain = EXP (ACT, from psum) -> fused (-1,min 0) TS (DVE)
     -> max TT (DVE, psum operand), writing bf16.
  2. v2[h, j*8+o] : per o: WT_o.T @ hrT -> psum [h, o*128+j]; strided
     casts write the j-major/o-fast INTERLEAVED bf16 layout (og0 ACT,
     og1 DVE).  With this order, main-output partition p has o = p%8 for
     EVERY chunk, so one shared [128,1] bias AP serves all evictions.
  3. TRANSPOSED main: psum[j*8+o - block, i] = v2_c.T @ hlT, 8 matmuls
     N=512 into 2-bank [128,1024] psum pairs.
  4. Evictions = one fused copy+bias+bf16-cast per group (ACT activation
     with bias / DVE tensor_scalar), grouped [c0c1][c2c3][c4c5][c6][c7]
     across ACT/DVE; out DMAs issue per group: first four on sync, the
     last on scalar right after ACT's c7 eviction.
  5. out DRAM [128, 4096] bf16: out_d[p, c*512+i] with p = jr*8+o,
     j = 16c+jr.  Host upcasts + unshuffles.

walrus's per-instruction HW structs carry at most ONE sync wait; a post-pass
splits multi-wait instructions into single-wait EventSemaphore predecessors.
"""

import numpy as np

import concourse.bass as bass
import concourse.mybir as mybir
import concourse.tile as tile
from concourse.bass_utils import run_bass_kernel_spmd

F32 = mybir.dt.float32
BF16 = mybir.dt.bfloat16

B = 2
N = 512
NIN = 64
H = 128
O = 8
JC = 128  # j-chunk per core
N_CORES = 8

# D1x packed-column offsets (bf16 elements)
_XLJ = 0              # xljT  [128]   (rows 0:64)
_XRH = 128            # xrhT  [130]   (rows 0:64)
_BRR = 258            # fc_r_b as a row on partition 0  [128]
_BLR = 386            # fc_l_b as a row on partition 0  [128]
_OBI = 514            # out-bias bb[p%8] per partition (f32 bitcast, 2 cols)
_D1XW = 516
_D1BW = 514           # xlhT (rows 64:128)

N_WARM = 11


def build_nc():
    nc = bass.Bass("TRN2")

    D1w = nc.dram_tensor("D1w", [128, 3 * H], BF16, kind="ExternalInput")
    D1x = nc.dram_tensor("D1x", [128, _D1XW], BF16, kind="ExternalInput")
    D1b = nc.dram_tensor("D1b", [128, _D1BW], BF16, kind="ExternalInput")
    Wt = nc.dram_tensor("Wt", [128, O * H], BF16, kind="ExternalInput")
    out_d = nc.dram_tensor("out", [128, O * N], BF16, kind="ExternalOutput")

    with tile.TileContext(nc) as tc:
        with (
            tc.tile_pool(name="persist", bufs=1) as pp,
            tc.tile_pool(name="ps_l1", bufs=1, space="PSUM") as ps_l1,
            tc.tile_pool(name="ps_v", bufs=1, space="PSUM") as ps_v,
            tc.tile_pool(name="ps_main", bufs=2, space="PSUM") as ps_main,
        ):
            D1w_sb = pp.tile([128, 3 * H], BF16, name="D1w_sb")
            D1x_sb = pp.tile([128, _D1XW], BF16, name="D1x_sb")
            D1b_sb = pp.tile([128, _D1BW], BF16, name="D1b_sb")
            W_sb = pp.tile([128, O * H], BF16, name="W_sb")
            warm_sb = pp.tile([128, 384], BF16, name="warm_sb")
            ones_sb = pp.tile([1, N], BF16, name="ones_sb")
            td = pp.tile([1, 2], F32, name="td")

            # ---- input DMAs on three parallel paths ----
            nc.sync.dma_start(D1w_sb[:], D1w[:])
            nc.sync.dma_start(W_sb[:], Wt[:])
            nc.scalar.dma_start(D1x_sb[:], D1x[:])
            nc.gpsimd.dma_start(D1b_sb[:], D1b[:])

            # ---- PE warm-up + early ACT table load ----
            nc.vector.memset(warm_sb[:], 0.0)
            nc.vector.memset(ones_sb[:], 1.0)
            nc.vector.memset(td[0:1, 0:1], 0.0)
            nc.scalar.activation(td[0:1, 1:2], td[0:1, 0:1],
                                 mybir.ActivationFunctionType.Exp)

            ps_hr = ps_l1.tile([128, JC], F32, name="ps_hr")
            ps_hl = ps_l1.tile([128, N], F32, name="ps_hl")
            for _ in range(N_WARM):
                nc.tensor.matmul(
                    ps_hl[:, 0:256], warm_sb[:, 0:128], warm_sb[:, 128:384],
                    start=True, stop=True,
                )

            # ---- layer 1 matmuls; K=1 rank-1 matmul adds the fc bias ----
            rhs_r = [
                D1x_sb[:, _XLJ:_XLJ + JC],           # x_l[j]
                D1x_sb[:, _XRH + 2:_XRH + 2 + JC],   # x_r[j+1] (bwd)
                D1x_sb[:, _XRH:_XRH + JC],           # x_r[j-1] (fwd)
            ]
            for c in range(3):
                nc.tensor.matmul(
                    ps_hr[:], D1w_sb[:, c * H:(c + 1) * H],
                    rhs_r[c], start=(c == 0), stop=False,
                )
            nc.tensor.matmul(ps_hr[:], D1x_sb[0:1, _BRR:_BRR + H],
                             ones_sb[0:1, 0:JC], start=False, stop=True)

            rhs_l = [
                D1b_sb[:, 1:1 + N],    # x_l[i]
                D1b_sb[:, 0:N],        # x_l[i-1] (fwd)
                D1b_sb[:, 2:2 + N],    # x_l[i+1] (bwd)
            ]
            for c in range(3):
                nc.tensor.matmul(
                    ps_hl[:], D1w_sb[:, c * H:(c + 1) * H],
                    rhs_l[c], start=(c == 0), stop=False,
                )
            nc.tensor.matmul(ps_hl[:], D1x_sb[0:1, _BLR:_BLR + H],
                             ones_sb[0:1, 0:N], start=False, stop=True)

            # ---- hr celu: e (ACT) -> TS min (DVE) -> TT max (DVE) ----
            hrT = pp.tile([128, JC], BF16, name="hrT")
            e_r = pp.tile([128, JC], F32, name="e_r")
            nc.scalar.activation(e_r[:], ps_hr[:],
                                 mybir.ActivationFunctionType.Exp)
            nc.vector.tensor_scalar(e_r[:], e_r[:], -1.0, 0.0,
                                    mybir.AluOpType.add, mybir.AluOpType.min)
            nc.vector.tensor_tensor(hrT[:], ps_hr[:], e_r[:],
                                    mybir.AluOpType.max)

            # ---- v2 matmuls: psum [h, (o,j)] per og group ----
            ps_v0 = ps_v.tile([128, 512], F32, name="ps_v0")
            ps_v1 = ps_v.tile([128, 512], F32, name="ps_v1")
            for og, ps_vo in ((0, ps_v0), (1, ps_v1)):
                for ol in range(4):
                    o = og * 4 + ol
                    nc.tensor.matmul(
                        ps_vo[:, ol * JC:(ol + 1) * JC],
                        W_sb[:, o * H:(o + 1) * H], hrT[:],
                        start=True, stop=True,
                    )

            # ---- hl celu in halves ----
            hlT = pp.tile([128, N], BF16, name="hlT")
            e_l = pp.tile([128, N], F32, name="e_l")
            for ch in range(2):
                sl = slice(ch * 256, (ch + 1) * 256)
                nc.scalar.activation(e_l[:, sl], ps_hl[:, sl],
                                     mybir.ActivationFunctionType.Exp)
            for ch in range(2):
                sl = slice(ch * 256, (ch + 1) * 256)
                nc.vector.tensor_scalar(e_l[:, sl], e_l[:, sl], -1.0, 0.0,
                                        mybir.AluOpType.add,
                                        mybir.AluOpType.min)
                nc.vector.tensor_tensor(hlT[:, sl], ps_hl[:, sl], e_l[:, sl],
                                        mybir.AluOpType.max)

            # ---- v2 casts to interleaved bf16 layout: col = j*8 + o ----
            v2sb = pp.tile([128, O * H], BF16, name="v2sb")
            v2v = v2sb[:].rearrange("p (j o) -> p j o", o=8)
            nc.scalar.copy(
                v2v[:, :, 0:4],
                ps_v0[:].rearrange("p (o j) -> p j o", o=4))
            nc.vector.tensor_copy(
                v2v[:, :, 4:8],
                ps_v1[:].rearrange("p (o j) -> p j o", o=4))

            # ---- main (transposed): psum[jo-block, i] = v2_c.T @ hlT ----
            # chunk c partition p -> j = 16c + p//8, o = p%8; the shared
            # per-partition bias bb[p%8] is added during eviction.
            out_sb = pp.tile([128, O * N], BF16, name="out_sb")
            ob_ap = D1x_sb[:, _OBI:_OBI + 2].bitcast(F32)
            ps_pairs = []
            for pr in range(4):
                ps_m = ps_main.tile([128, 2 * N], F32, name="ps_m")
                ps_pairs.append(ps_m)
                for half in range(2):
                    c = 2 * pr + half
                    nc.tensor.matmul(
                        ps_m[:, half * N:(half + 1) * N],
                        v2sb[:, c * JC:(c + 1) * JC], hlT[:],
                        start=True, stop=True,
                    )

            def evict(eng, pr, col0, col1):
                src = ps_pairs[pr][:, col0 - pr * 1024:col1 - pr * 1024]
                dst = out_sb[:, col0:col1]
                if eng is nc.scalar:
                    nc.scalar.activation(dst, src,
                                         mybir.ActivationFunctionType.Identity,
                                         bias=ob_ap, scale=1.0)
                else:
                    nc.vector.tensor_scalar_add(dst, src, ob_ap)

            # groups: ACT [c0c1], DVE [c2c3], ACT [c4c5], DVE [c6], ACT [c7]
            evict(nc.scalar, 0, 0, 1024)
            nc.sync.dma_start(out_d[:, 0:1024], out_sb[:, 0:1024])
            evict(nc.vector, 1, 1024, 2048)
            nc.sync.dma_start(out_d[:, 1024:2048], out_sb[:, 1024:2048])
            evict(nc.scalar, 2, 2048, 3072)
            nc.sync.dma_start(out_d[:, 2048:3072], out_sb[:, 2048:3072])
            evict(nc.vector, 3, 3072, 3584)
            nc.sync.dma_start(out_d[:, 3072:3584], out_sb[:, 3072:3584])
            evict(nc.scalar, 3, 3584, 4096)
            nc.scalar.dma_start(out_d[:, 3584:4096], out_sb[:, 3584:4096])

    _legalize_waits(nc)
    return nc


def _legalize_waits(nc):
    """walrus's per-instruction HW structs carry at most ONE sync wait.
    Split any instruction with >1 on_wait into same-engine single-wait
    EventSemaphore predecessors (engine executes them in program order)."""
    n = 0
    for bb in nc.main_func.blocks:
        insts = list(bb.instructions)
        out = []
        for ins in insts:
            si = ins.sync_info
            waits = list(si.on_wait) if si and si.on_wait else []
            if len(waits) > 1:
                for w in waits[:-1]:
                    n += 1
                    out.append(mybir.InstEventSemaphore(
                        name=f"wait-split-{n}",
                        opcode="EventSemaphore",
                        engine=ins.engine,
                        ins=[], outs=[],
                        sync_info=mybir.SyncInfo(on_wait=[w], on_update=[]),
                    ))
                si.on_wait = [waits[-1]]
            out.append(ins)
        if n:
            bb.instructions = out
    return nc


_NC_CACHE = None


def _get_nc():
    global _NC_CACHE
    if _NC_CACHE is None:
        _NC_CACHE = build_nc()
    return _NC_CACHE


def _prep_core_inputs(x_l, x_r, fc_l_W, fc_l_b, fc_r_W, fc_r_b, bilinear_W, bilinear_b):
    """Host-side sharding: build the 8 per-core input dicts."""
    import ml_dtypes

    f32 = np.float32
    bf16 = ml_dtypes.bfloat16
    x_l = np.ascontiguousarray(x_l, f32)
    x_r = np.ascontiguousarray(x_r, f32)

    # WT[g, o*H + h] = W[o, h, g]
    WT = np.ascontiguousarray(
        np.asarray(bilinear_W, f32).transpose(2, 0, 1).reshape(128, O * H)
    ).astype(bf16)

    D1w = np.zeros((128, 3 * H), bf16)
    frW = np.asarray(fc_r_W, f32)
    flW = np.asarray(fc_l_W, f32)
    for c in range(3):
        D1w[:NIN, c * H:(c + 1) * H] = frW[:, c * NIN:(c + 1) * NIN].T.astype(bf16)
        D1w[NIN:, c * H:(c + 1) * H] = flW[:, c * NIN:(c + 1) * NIN].T.astype(bf16)

    D1x_c = np.zeros((128, _D1XW), bf16)
    D1x_c[0, _BRR:_BRR + H] = np.asarray(fc_r_b, f32).astype(bf16)
    D1x_c[0, _BLR:_BLR + H] = np.asarray(fc_l_b, f32).astype(bf16)
    obi = np.asarray(bilinear_b, f32)[np.arange(128) % O]  # bb[p%8]
    D1x_c.view(np.uint16)[:, _OBI:_OBI + 2] = obi.reshape(-1, 1).view('<u2')

    # D1b per batch: xlhT rows 64:128, col t = x_l[b, t-1]
    D1bs = []
    for b in range(B):
        D1b = np.zeros((128, _D1BW), bf16)
        D1b[NIN:, 1:1 + N] = x_l[b].T.astype(bf16)
        D1bs.append(D1b)

    in_maps = []
    for core in range(N_CORES):
        b, jg = core // 4, core % 4
        j0 = jg * JC
        D1x = D1x_c.copy()
        D1x[:NIN, _XLJ:_XLJ + JC] = x_l[b, j0:j0 + JC].T.astype(bf16)
        # xrhT: col t = x_r[b, j0-1+t], zero-padded at global edges
        lo = max(j0 - 1, 0)
        hi = min(j0 + JC + 1, N)
        D1x[:NIN, _XRH + lo - (j0 - 1):_XRH + hi - (j0 - 1)] = \
            x_r[b, lo:hi].T.astype(bf16)
        in_maps.append({"D1w": D1w, "D1x": D1x, "D1b": D1bs[b], "Wt": WT})
    return in_maps


def _run(inputs, trace=False, **kw):
    nc = _get_nc()
    in_maps = _prep_core_inputs(**inputs)
    res = run_bass_kernel_spmd(
        nc, in_maps, core_ids=list(range(N_CORES)), trace=trace, **kw)
    out = np.empty((B, N, N, O), np.float32)
    for core in range(N_CORES):
        b, jg = core // 4, core % 4
        j0 = jg * JC
        # device out: [p = jr*8+o, c*512 + i] -> out[i, 16c+jr, o]
        arr = np.asarray(res.results[core]["out"]).astype(np.float32)
        arr = arr.reshape(16, 8, 8, N)          # [jr, o, c, i]
        out[b, :, j0:j0 + JC, :] = \
            arr.transpose(3, 2, 0, 1).reshape(N, JC, O)
    return out, res


def kernel(**inputs):
    out, _ = _run(inputs, trace=False)
    return out
